# revision 1
# baseline (speedup 1.0000x reference)
"""Trainium2 Bass kernel for nn_LocalExperts (MoE expert-parallel FFN).

Reference computation (per full input):
    x  [T=16384, D=1024] -> reshape [E=8, C=2048, D]
    h  = gelu(x @ w1[e] + b1[e])     w1 [E, D, F=4096]
    y  = h @ w2[e] + b2[e]           w2 [E, F, D]
    out[T, D]

Sharding: expert parallelism across 8 NeuronCores. Expert e's tokens are
exactly rows [e*C:(e+1)*C] of the input, so core e gets that token slice
plus w1[e], b1[e], w2[e], b2[e]. No collectives needed; outputs are
concatenated on the host.

Per-core kernel (C=2048 tokens, one expert):
  - X is PE-transposed once into Xt (contraction dim D on partitions).
  - Two token passes of CP=1024; per pass, loop F in chunks of FC=512:
      GEMM1: Ht[f,c] = gelu(W1c.T-tiles @ Xt + b1)   (PSUM acc over D)
      GEMM2: Yacc[c,d] += Ht-tiles.T @ W2c           (PSUM acc over FC,
                                                      DVE acc over chunks)
  - Yacc initialized with broadcast b2 on the first chunk.
  - Matmuls run as float32r (full PE rate at N=512, ~TF32 precision,
    fp32 PSUM accumulation).
"""

import os
from contextlib import ExitStack

import numpy as np

import concourse.bass as bass
import concourse.tile as tile
from concourse import bacc
from concourse import mybir
from concourse.bass import ds, ts
from concourse.bass_utils import run_bass_kernel_spmd
from concourse.masks import make_identity

AFT = mybir.ActivationFunctionType

E = 8
D = 1024
F = 4096
T = 16384
C = T // E          # tokens per core
P = 128

N_PASS = 2          # token passes (halves SBUF residency of Xt/Yacc)
CP = C // N_PASS    # tokens per pass
FC = 512            # F chunk per iteration
NFREE = 512         # matmul moving free dim (one PSUM bank of fp32)

# "f32r" (default): fp32 data, float32r matmul (full PE rate, ~TF32 mantissa)
# "f32": plain fp32 matmul (4 cycles/row, ~4x slower PE)
# "bf16": cast operands to bf16 (full rate, FWL weight loads)
MM_MODE = os.environ.get("KERNEL_MM_MODE", "f32r")
# test-only: CoreSim lacks Gelu; "tanh" swaps the activation for sim gating
ACT_FN = os.environ.get("KERNEL_ACT", "gelu")


def _emit(ctx: ExitStack, tc: tile.TileContext, x, w1, b1, w2, b2, y):
    nc = tc.nc
    f32 = mybir.dt.float32
    bf16 = mybir.dt.bfloat16
    f32r = mybir.dt.float32r
    # dtype of matmul operand tiles in SBUF. The BIR verifier requires fp32r
    # matmul inputs to be WRITTEN as float32r by their producer (DVE/ACT
    # conversion rounds), so Xt/Ht are allocated natively in this dtype and
    # w1/w2 arrive from DRAM tensors declared float32r.
    mm_sb_dt = {"bf16": bf16, "f32r": f32r, "f32": f32}[MM_MODE]

    consts = ctx.enter_context(tc.tile_pool(name="consts", bufs=1))
    xstage = ctx.enter_context(tc.tile_pool(name="xstage", bufs=4))
    xt_pool = ctx.enter_context(tc.tile_pool(name="xt", bufs=1))
    yacc_pool = ctx.enter_context(tc.tile_pool(name="yacc", bufs=1))
    w1_pool = ctx.enter_context(tc.tile_pool(name="w1c", bufs=2))
    w2_pool = ctx.enter_context(tc.tile_pool(name="w2c", bufs=2))
    ht_pool = ctx.enter_context(tc.tile_pool(name="ht", bufs=2))
    mm_psum = ctx.enter_context(tc.tile_pool(name="mmp", bufs=8, space="PSUM"))
    if MM_MODE == "bf16":
        wstage = ctx.enter_context(tc.tile_pool(name="wstage", bufs=2))

    identity = consts.tile([P, P], f32)
    make_identity(nc, identity[:])

    # b1 wants layout [p, fo] (column ft = per-partition bias of f-tile ft).
    # A direct strided DMA is 4096 4-byte packets; instead DMA the contiguous
    # [fo, p] form (32 partitions x 512B) and PE-transpose it.
    F_T = F // P
    b1row = consts.tile([F_T, P], f32)
    b1t = consts.tile([P, F_T], f32)
    # b2 broadcast across partitions for the Yacc init
    b2b = consts.tile([P, D], f32)

    def load_consts():
        nc.scalar.dma_start(b1row[:], b1.rearrange("(fo p) -> fo p", p=P))
        bp = mm_psum.tile([P, NFREE], f32, tag="mm")
        nc.tensor.transpose(bp[:, :F_T], b1row[:], identity[:F_T, :F_T])
        nc.any.tensor_copy(out=b1t[:], in_=bp[:, :F_T])
        nc.scalar.dma_start(b2b[:], b2[None, :].to_broadcast((P, D)))

    # Warm the PE HAM clock (cold 1.2GHz -> 2.4GHz needs ~3.4us of activity)
    # during the initial X-row DMA wait, using identity matmuls.
    warm_ps = mm_psum.tile([P, NFREE], f32, tag="mm")
    for _ in range(24):
        nc.tensor.matmul(warm_ps[:, :P], lhsT=identity[:], rhs=identity[:],
                         start=True, stop=True)

    w1_r = w1.rearrange("(do p) f -> p do f", p=P)    # [128, 8, 4096]
    w2_r = w2.rearrange("(fo p) d -> p fo d", p=P)    # [128, 32, 1024]

    D_T = D // P        # 8 d-tiles
    FC_T = FC // P      # 4 f-tiles per chunk
    N_FC = F // FC      # 8 chunks

    xrows_next = []
    for pss in range(N_PASS):
        c_base = pss * CP

        # ---- transpose X[c_base:c_base+CP, :] into Xt [128(d), D_T, CP] ----
        xt = xt_pool.tile([P, D_T, CP], mm_sb_dt, tag="xt")
        wpre = None
        for ci in range(CP // P):
            if pss == 1 and ci < len(xrows_next):
                xrow = xrows_next[ci]  # prefetched during pass 0
            else:
                xrow = xstage.tile([P, D], f32, tag="xrow")
                nc.sync.dma_start(xrow[:], x[ds(c_base + ci * P, P), :])
            for di in range(D_T):
                pt = mm_psum.tile([P, NFREE], f32, tag="mm")
                nc.tensor.transpose(pt[:, :P], xrow[:, ds(di * P, P)], identity[:])
                nc.any.tensor_copy(out=xt[:, di, ds(ci * P, P)], in_=pt[:, :P])
            # chunk-0 weight prefetch, 512KB slices interleaved between X rows
            if ci == 0:
                w1c0 = w1_pool.tile([P, D_T, FC], mm_sb_dt, tag="w1c")
                w2c0 = w2_pool.tile([P, FC_T, D], mm_sb_dt, tag="w2c")
                wpre = (w1c0, w2c0)
            if ci < 4:
                nc.sync.dma_start(
                    w1c0[:, ds(2 * ci, 2), :], w1_r[:, ds(2 * ci, 2), ds(0, FC)]
                )
            else:
                nc.sync.dma_start(
                    w2c0[:, ds(ci - 4, 1), :], w2_r[:, ds(ci - 4, 1), :]
                )
            if pss == 0 and ci == 4:
                load_consts()

        yacc = yacc_pool.tile([P, CP // P, D], f32, tag="yacc")

        for fci in range(N_FC):
            f0 = fci * FC

            # ---- weight chunks ----
            if MM_MODE == "bf16":
                w1s = wstage.tile([P, D_T, FC], f32, tag="w1s")
                nc.sync.dma_start(w1s[:], w1_r[:, :, ds(f0, FC)])
                w1c = w1_pool.tile([P, D_T, FC], bf16, tag="w1c")
                nc.vector.tensor_copy(out=w1c[:], in_=w1s[:])
                w2s = wstage.tile([P, FC_T, D], f32, tag="w2s")
                nc.sync.dma_start(w2s[:], w2_r[:, ds(fci * FC_T, FC_T), :])
                w2c = w2_pool.tile([P, FC_T, D], bf16, tag="w2c")
                nc.vector.tensor_copy(out=w2c[:], in_=w2s[:])
            else:
                # w1/w2 DRAM tensors are declared mm_sb_dt; direct DMA
                if fci == 0:
                    w1c, w2c = wpre
                else:
                    w1c = w1_pool.tile([P, D_T, FC], mm_sb_dt, tag="w1c")
                    nc.sync.dma_start(w1c[:], w1_r[:, :, ds(f0, FC)])
                    w2c = w2_pool.tile([P, FC_T, D], mm_sb_dt, tag="w2c")
                    nc.sync.dma_start(w2c[:], w2_r[:, ds(fci * FC_T, FC_T), :])
                if pss == 0 and 1 <= fci <= 4:
                    # prefetch next pass's X row fci-1 (xstage slot is long
                    # free; avoids head-of-line stall at the pass boundary)
                    nci = fci - 1
                    xr = xstage.tile([P, D], f32, tag="xrow")
                    nc.sync.dma_start(xr[:], x[ds(CP + nci * P, P), :])
                    xrows_next.append(xr)

            # ---- GEMM1: Ht[f, c] = gelu(sum_d W1[d, f]^T X^T[d, c] + b1[f]) ----
            ht = ht_pool.tile([P, FC_T, CP], mm_sb_dt, tag="ht")
            for fti in range(FC_T):
                ft_g = fci * FC_T + fti
                for cci in range(CP // NFREE):
                    ps = mm_psum.tile([P, NFREE], f32, tag="mm")
                    for di in range(D_T):
                        nc.tensor.matmul(
                            ps[:],
                            lhsT=w1c[:, di, ds(fti * P, P)],
                            rhs=xt[:, di, ds(cci * NFREE, NFREE)],
                            start=(di == 0),
                            stop=(di == D_T - 1),
                        )
                    nc.scalar.activation(
                        ht[:, fti, ds(cci * NFREE, NFREE)],
                        ps[:],
                        AFT.Tanh if ACT_FN == "tanh" else AFT.Gelu_apprx_tanh,
                        bias=b1t[:, ft_g : ft_g + 1],
                        scale=1.0,
                    )

            # ---- GEMM2: Yacc[c, d] += sum_f Ht[f, c]^T W2[f, d] ----
            for ci in range(CP // P):
                for dci in range(D // NFREE):
                    ps = mm_psum.tile([P, NFREE], f32, tag="mm")
                    for fti in range(FC_T):
                        nc.tensor.matmul(
                            ps[:],
                            lhsT=ht[:, fti, ds(ci * P, P)],
                            rhs=w2c[:, fti, ds(dci * NFREE, NFREE)],
                            start=(fti == 0),
                            stop=(fti == FC_T - 1),
                        )
                    ya = yacc[:, ci, ds(dci * NFREE, NFREE)]
                    if fci == 0:
                        nc.vector.tensor_add(
                            out=ya, in0=ps[:], in1=b2b[:, ds(dci * NFREE, NFREE)]
                        )
                    else:
                        nc.vector.tensor_add(out=ya, in0=ya, in1=ps[:])
                if fci == N_FC - 1:
                    # row complete: writeback from the ACT queue (idle during
                    # GEMM2 phases; keeps sync free for X-row DMAs)
                    nc.scalar.dma_start(y[ds(c_base + ci * P, P), :], yacc[:, ci, :])


_NC_CACHE = None


def build_bass():
    global _NC_CACHE
    if _NC_CACHE is not None:
        return _NC_CACHE
    nc = bacc.Bacc("TRN2", target_bir_lowering=False, debug=False)
    f32 = mybir.dt.float32
    w_dt = mybir.dt.float32r if MM_MODE == "f32r" else f32
    x = nc.dram_tensor("x", [C, D], f32, kind="ExternalInput").ap()
    w1 = nc.dram_tensor("w1", [D, F], w_dt, kind="ExternalInput").ap()
    b1 = nc.dram_tensor("b1", [F], f32, kind="ExternalInput").ap()
    w2 = nc.dram_tensor("w2", [F, D], w_dt, kind="ExternalInput").ap()
    b2 = nc.dram_tensor("b2", [D], f32, kind="ExternalInput").ap()
    y = nc.dram_tensor("y", [C, D], f32, kind="ExternalOutput").ap()
    with tile.TileContext(nc) as tc:
        with ExitStack() as ctx:
            _emit(ctx, tc, x, w1, b1, w2, b2, y)
    nc.compile()
    _NC_CACHE = nc
    return nc


def _in_maps(inputs, w1, b1, w2, b2):
    return [
        {
            "x": np.ascontiguousarray(inputs[e * C : (e + 1) * C]),
            "w1": np.ascontiguousarray(w1[e]),
            "b1": np.ascontiguousarray(b1[e]),
            "w2": np.ascontiguousarray(w2[e]),
            "b2": np.ascontiguousarray(b2[e]),
        }
        for e in range(E)
    ]


def kernel_run(inputs, w1, b1, w2, b2, trace=False, **trace_kwargs):
    """Run on 8 NeuronCores; returns (full_output [T, D], BassKernelResults)."""
    inputs = np.asarray(inputs, dtype=np.float32)
    w1 = np.asarray(w1, dtype=np.float32)
    b1 = np.asarray(b1, dtype=np.float32)
    w2 = np.asarray(w2, dtype=np.float32)
    b2 = np.asarray(b2, dtype=np.float32)
    nc = build_bass()
    res = run_bass_kernel_spmd(
        nc,
        _in_maps(inputs, w1, b1, w2, b2),
        core_ids=list(range(E)),
        trace=trace,
        **trace_kwargs,
    )
    out = np.concatenate([res.results[e]["y"] for e in range(E)], axis=0)
    return out, res


def kernel(inputs, w1, b1, w2, b2):
    out, _ = kernel_run(inputs, w1, b1, w2, b2, trace=False)
    return out



# revision 6
# speedup vs baseline: 1.0758x; 1.0758x over previous
"""Trainium2 Bass kernel for nn_LocalExperts (MoE expert-parallel FFN), v2.

Reference computation (per full input):
    x  [T=16384, D=1024] -> reshape [E=8, C=2048, D]
    h  = gelu(x @ w1[e] + b1[e])     w1 [E, D, F=4096]
    y  = h @ w2[e] + b2[e]           w2 [E, F, D]
    out[T, D]

Sharding: expert parallelism across 8 NeuronCores. Expert e's tokens are
exactly rows [e*C:(e+1)*C] of the input, so core e gets that token slice
plus w1[e], b1[e], w2[e], b2[e]. No collectives; outputs are concatenated
on the host.

v2 design (vs v1's two token passes of f32r):
  - Host prep: x arrives pre-transposed [D, C] and cast to bf16; w1/w2
    cast to bf16; b1 pre-packed [128, F/128]. No PE transposes, no
    identity, and weight DMA halves (bf16). b1/b2/PSUM/y stay f32.
  - Single token pass, F chunked in 8 x 512: weights are DMA'd exactly
    once (20 MB total reads vs v1's 76 MB).
  - bf16 matmuls get the compiler's fast-weight-load: LDWEIGHTS hides
    under the 512-row stream, unlike f32r's exposed ~187 ns loads.
  - Per chunk: GEMM1 (q-outer so chunk 0 can start as soon as the first
    quarter of Xt lands, ~4 us in), then GEMM2 starting on token blocks
    whose ht slices drained long ago -> the PE never waits on ACT.
  - Yacc [C, D] f32 accumulates GEMM2 partials on DVE; last chunk's
    drain triggers the row writeback on the ACT queue.
"""

import os
from contextlib import ExitStack

import ml_dtypes
import numpy as np

import concourse.bass as bass
import concourse.tile as tile
from concourse import bacc
from concourse import mybir
from concourse.bass import ds, ts
from concourse.bass_utils import run_bass_kernel_spmd

AFT = mybir.ActivationFunctionType

E = 8
D = 1024
F = 4096
T = 16384
C = T // E          # tokens per core
P = 128

FC = 512            # F chunk per iteration
N_FC = F // FC      # 8 chunks
FC_T = FC // P      # 4 f-tiles per chunk
D_T = D // P        # 8 d-tiles
C_B = C // P        # 16 token blocks
NQ = 4              # token quarters for GEMM1 moving dim
QW = C // NQ        # 512 tokens per quarter
N_WARM = 24         # HAM clock warm-up matmuls

# test-only: CoreSim lacks Gelu; "tanh" swaps the activation for sim gating
ACT_FN = os.environ.get("KERNEL_ACT", "gelu")


def _emit(ctx: ExitStack, tc: tile.TileContext, xT, w1, b1t_d, w2, b2, y):
    nc = tc.nc
    f32 = mybir.dt.float32
    bf16 = mybir.dt.bfloat16

    consts = ctx.enter_context(tc.tile_pool(name="consts", bufs=1))
    xt_pool = ctx.enter_context(tc.tile_pool(name="xt", bufs=1))
    yacc_pool = ctx.enter_context(tc.tile_pool(name="yacc", bufs=1))
    w1_pool = ctx.enter_context(tc.tile_pool(name="w1c", bufs=2))
    w2_pool = ctx.enter_context(tc.tile_pool(name="w2c", bufs=2))
    ht_pool = ctx.enter_context(tc.tile_pool(name="ht", bufs=2))
    mm1_psum = ctx.enter_context(tc.tile_pool(name="mm1", bufs=2, space="PSUM"))
    mm2_psum = ctx.enter_context(tc.tile_pool(name="mm2", bufs=4, space="PSUM"))
    warm_psum = ctx.enter_context(tc.tile_pool(name="wrm", bufs=1, space="PSUM"))

    # Warm the PE HAM clock (cold 1.2GHz -> 2.4GHz needs ~3.4us of activity)
    # while the first Xt quarter + w1 chunk DMA in.
    dummy = consts.tile([P, QW], bf16)
    nc.gpsimd.memset(dummy[:], 0.0)
    warm_ps = warm_psum.tile([P, QW], f32, tag="warm")
    for _ in range(N_WARM):
        nc.tensor.matmul(warm_ps[:], lhsT=dummy[:, :P], rhs=dummy[:],
                         start=True, stop=True)

    # ---- staged inputs ----
    xT_r = xT.rearrange("(do p) c -> p do c", p=P)    # [128, 8, 2048]
    w1_r = w1.rearrange("(do p) f -> p do f", p=P)    # [128, 8, 4096]
    w2_r = w2.rearrange("(fo p) d -> p fo d", p=P)    # [128, 32, 1024]

    xt = xt_pool.tile([P, D_T, C], bf16, tag="xt")
    yacc = yacc_pool.tile([P, C_B, D], f32, tag="yacc")
    b1t = consts.tile([P, F // P], f32)
    b2b = consts.tile([P, D], f32)

    w1cs = [None] * N_FC
    w2cs = [None] * N_FC

    def load_chunk(k, q):
        w1cs[k] = w1_pool.tile([P, D_T, FC], bf16, tag="w1c", name=f"w1c{k}")
        w2cs[k] = w2_pool.tile([P, FC_T, D], bf16, tag="w2c", name=f"w2c{k}")
        q.dma_start(w1cs[k][:], w1_r[:, :, ds(k * FC, FC)])
        q.dma_start(w2cs[k][:], w2_r[:, ds(k * FC_T, FC_T), :])

    # startup DMAs: sync queue feeds Xt quarters 0/1 + chunk-0 weights;
    # scalar queue feeds quarters 2/3 + consts (GEMM1 needs q0+w1c0 first).
    nc.sync.dma_start(xt[:, :, ds(0, QW)], xT_r[:, :, ds(0, QW)])
    load_chunk(0, nc.scalar)
    nc.sync.dma_start(xt[:, :, ds(QW, QW)], xT_r[:, :, ds(QW, QW)])
    nc.scalar.dma_start(xt[:, :, ds(2 * QW, QW)], xT_r[:, :, ds(2 * QW, QW)])
    nc.scalar.dma_start(xt[:, :, ds(3 * QW, QW)], xT_r[:, :, ds(3 * QW, QW)])
    nc.scalar.dma_start(b1t[:], b1t_d)
    nc.scalar.dma_start(b2b[:], b2[None, :].to_broadcast((P, D)))

    act_fn = AFT.Tanh if ACT_FN == "tanh" else AFT.Gelu_apprx_tanh

    for k in range(N_FC):
        # prefetch next chunk's weights early in this chunk's compute
        if k + 1 < N_FC:
            load_chunk(k + 1, nc.sync)
        w1c, w2c = w1cs[k], w2cs[k]
        w1cs[k] = w2cs[k] = None

        # ---- GEMM1: ht[f, c] = gelu(sum_d W1[d, f]^T Xt[d, c] + b1[f]) ----
        ht = ht_pool.tile([P, FC_T, C], bf16, tag="ht")
        for qi in range(NQ):
            for fti in range(FC_T):
                ps = mm1_psum.tile([P, QW], f32, tag="mm1")
                for di in range(D_T):
                    nc.tensor.matmul(
                        ps[:],
                        lhsT=w1c[:, di, ds(fti * P, P)],
                        rhs=xt[:, di, ds(qi * QW, QW)],
                        start=(di == 0),
                        stop=(di == D_T - 1),
                    )
                ft_g = k * FC_T + fti
                nc.scalar.activation(
                    ht[:, fti, ds(qi * QW, QW)],
                    ps[:],
                    act_fn,
                    bias=b1t[:, ft_g : ft_g + 1],
                    scale=1.0,
                )

        # ---- GEMM2: Yacc[c, d] += sum_f ht[f, c]^T W2[f, d] ----
        for ci in range(C_B):
            for dci in range(2):
                ps = mm2_psum.tile([P, QW], f32, tag="mm2")
                for fti in range(FC_T):
                    nc.tensor.matmul(
                        ps[:],
                        lhsT=ht[:, fti, ds(ci * P, P)],
                        rhs=w2c[:, fti, ds(dci * QW, QW)],
                        start=(fti == 0),
                        stop=(fti == FC_T - 1),
                    )
                ya = yacc[:, ci, ds(dci * QW, QW)]
                if k == 0:
                    nc.vector.tensor_add(out=ya, in0=ps[:], in1=b2b[:, ds(dci * QW, QW)])
                else:
                    nc.vector.tensor_add(out=ya, in0=ya, in1=ps[:])
            if k == N_FC - 1:
                # row complete: writeback from the ACT queue (idle during
                # GEMM2 phases)
                nc.scalar.dma_start(y[ds(ci * P, P), :], yacc[:, ci, :])


_NC_CACHE = None


def build_bass():
    global _NC_CACHE
    if _NC_CACHE is not None:
        return _NC_CACHE
    nc = bacc.Bacc("TRN2", target_bir_lowering=False, debug=False)
    f32 = mybir.dt.float32
    bf16 = mybir.dt.bfloat16
    xT = nc.dram_tensor("xT", [D, C], bf16, kind="ExternalInput").ap()
    w1 = nc.dram_tensor("w1", [D, F], bf16, kind="ExternalInput").ap()
    b1t = nc.dram_tensor("b1t", [P, F // P], f32, kind="ExternalInput").ap()
    w2 = nc.dram_tensor("w2", [F, D], bf16, kind="ExternalInput").ap()
    b2 = nc.dram_tensor("b2", [D], f32, kind="ExternalInput").ap()
    y = nc.dram_tensor("y", [C, D], f32, kind="ExternalOutput").ap()
    with tile.TileContext(nc) as tc:
        with ExitStack() as ctx:
            _emit(ctx, tc, xT, w1, b1t, w2, b2, y)
    nc.compile()
    _NC_CACHE = nc
    return nc


def _in_maps(inputs, w1, b1, w2, b2):
    bf = ml_dtypes.bfloat16
    maps = []
    for e in range(E):
        xs = inputs[e * C : (e + 1) * C]
        maps.append(
            {
                "xT": np.ascontiguousarray(xs.T).astype(bf),
                "w1": w1[e].astype(bf),
                "b1t": np.ascontiguousarray(
                    b1[e].reshape(F // P, P).T.astype(np.float32)
                ),
                "w2": w2[e].astype(bf),
                "b2": np.ascontiguousarray(b2[e], dtype=np.float32),
            }
        )
    return maps


def kernel_run(inputs, w1, b1, w2, b2, trace=False, **trace_kwargs):
    """Run on 8 NeuronCores; returns (full_output [T, D], BassKernelResults)."""
    inputs = np.asarray(inputs, dtype=np.float32)
    w1 = np.asarray(w1, dtype=np.float32)
    b1 = np.asarray(b1, dtype=np.float32)
    w2 = np.asarray(w2, dtype=np.float32)
    b2 = np.asarray(b2, dtype=np.float32)
    nc = build_bass()
    res = run_bass_kernel_spmd(
        nc,
        _in_maps(inputs, w1, b1, w2, b2),
        core_ids=list(range(E)),
        trace=trace,
        **trace_kwargs,
    )
    out = np.concatenate([res.results[e]["y"] for e in range(E)], axis=0)
    return out, res


def kernel(inputs, w1, b1, w2, b2):
    out, _ = kernel_run(inputs, w1, b1, w2, b2, trace=False)
    return out


# revision 13
# speedup vs baseline: 1.1218x; 1.0428x over previous
"""Trainium2 Bass kernel for nn_LocalExperts (MoE expert-parallel FFN), v2.

Reference computation (per full input):
    x  [T=16384, D=1024] -> reshape [E=8, C=2048, D]
    h  = gelu(x @ w1[e] + b1[e])     w1 [E, D, F=4096]
    y  = h @ w2[e] + b2[e]           w2 [E, F, D]
    out[T, D]

Sharding: expert parallelism across 8 NeuronCores. Expert e's tokens are
exactly rows [e*C:(e+1)*C] of the input, so core e gets that token slice
plus w1[e], b1[e], w2[e], b2[e]. No collectives; outputs are concatenated
on the host.

v2 design (vs v1's two token passes of f32r):
  - Host prep: x arrives pre-transposed [D, C] and cast to bf16; w1/w2
    cast to bf16; b1 pre-packed [128, F/128]. No PE transposes, no
    identity, and weight DMA halves (bf16). b1/b2/PSUM/y stay f32.
  - Single token pass, F chunked in 8 x 512: weights are DMA'd exactly
    once (20 MB total reads vs v1's 76 MB).
  - bf16 matmuls get the compiler's fast-weight-load: LDWEIGHTS hides
    under the 512-row stream, unlike f32r's exposed ~187 ns loads.
  - Per chunk: GEMM1 (q-outer so chunk 0 can start as soon as the first
    quarter of Xt lands, ~4 us in), then GEMM2 starting on token blocks
    whose ht slices drained long ago -> the PE never waits on ACT.
  - Yacc [C, D] f32 accumulates GEMM2 partials on DVE; last chunk's
    drain triggers the row writeback on the ACT queue.
"""

import os
from contextlib import ExitStack

import ml_dtypes
import numpy as np

import concourse.bass as bass
import concourse.tile as tile
from concourse import bacc
from concourse import mybir
from concourse.bass import ds, ts
from concourse.bass_utils import run_bass_kernel_spmd

AFT = mybir.ActivationFunctionType

E = 8
D = 1024
F = 4096
T = 16384
C = T // E          # tokens per core
P = 128

FC = 512            # F chunk per iteration
N_FC = F // FC      # 8 chunks
FC_T = FC // P      # 4 f-tiles per chunk
D_T = D // P        # 8 d-tiles
C_B = C // P        # 16 token blocks
NQ = 4              # token quarters for GEMM1 moving dim
QW = C // NQ        # 512 tokens per quarter
N_WARM = int(os.environ.get("KERNEL_WARM", "5"))  # HAM clock warm-up matmuls
# moving free dim 1024 (2 PSUM banks/matmul, halves instruction count);
# bf16 moving operand max is 128x1024 per the TRN2 ISA table
MV1024 = os.environ.get("KERNEL_MV1024", "0") == "1"

# test-only: CoreSim lacks Gelu; "tanh" swaps the activation for sim gating
ACT_FN = os.environ.get("KERNEL_ACT", "gelu")


def _emit(ctx: ExitStack, tc: tile.TileContext, xT, w1, b1t_d, w2, b2, y):
    nc = tc.nc
    f32 = mybir.dt.float32
    bf16 = mybir.dt.bfloat16

    consts = ctx.enter_context(tc.tile_pool(name="consts", bufs=1))
    xt_pool = ctx.enter_context(tc.tile_pool(name="xt", bufs=1))
    yacc_pool = ctx.enter_context(tc.tile_pool(name="yacc", bufs=1))
    w1_pool = ctx.enter_context(tc.tile_pool(name="w1c", bufs=2))
    w2_pool = ctx.enter_context(tc.tile_pool(name="w2c", bufs=2))
    ht_pool = ctx.enter_context(tc.tile_pool(name="ht", bufs=2))
    mm1_psum = ctx.enter_context(tc.tile_pool(name="mm1", bufs=2, space="PSUM"))
    mm2_psum = ctx.enter_context(
        tc.tile_pool(name="mm2", bufs=(2 if MV1024 else 4), space="PSUM")
    )

    # Warm the PE HAM clock (cold 1.2GHz -> 2.4GHz needs ~3.4us of activity)
    # while the first Xt slices + w1 chunk DMA in; the first real chain is
    # DMA-paced, so it continues the busy stream and finishes the ramp.
    dummy = consts.tile([P, QW], bf16)
    nc.gpsimd.memset(dummy[:], 0.0)
    for _ in range(N_WARM):
        warm_ps = mm1_psum.tile([P, QW], f32, tag="mm1", name="warm_ps")
        nc.tensor.matmul(warm_ps[:], lhsT=dummy[:, :P], rhs=dummy[:],
                         start=True, stop=True)

    # ---- staged inputs ----
    xT_r = xT.rearrange("(do p) c -> p do c", p=P)    # [128, 8, 2048]
    w1_r = w1.rearrange("(do p) f -> p do f", p=P)    # [128, 8, 4096]
    w2_r = w2.rearrange("(fo p) d -> p fo d", p=P)    # [128, 32, 1024]

    xt = xt_pool.tile([P, D_T, C], bf16, tag="xt")
    yacc = yacc_pool.tile([P, C_B, D], f32, tag="yacc")
    b1t = consts.tile([P, F // P], f32)
    b2b = consts.tile([P, D], f32)

    w1cs = [None] * N_FC
    w2cs = [None] * N_FC

    def load_chunk(k, q):
        w1cs[k] = w1_pool.tile([P, D_T, FC], bf16, tag="w1c", name=f"w1c{k}")
        w2cs[k] = w2_pool.tile([P, FC_T, D], bf16, tag="w2c", name=f"w2c{k}")
        q.dma_start(w1cs[k][:], w1_r[:, :, ds(k * FC, FC)])
        q.dma_start(w2cs[k][:], w2_r[:, ds(k * FC_T, FC_T), :])

    # Startup DMAs, ordered so the first GEMM1 chain (q0, f-tile 0) can
    # start ~2us in: tiny biases first, then w1 chunk 0's first f-tile,
    # then everything else. Xt quarter 0 lands in per-di slices so the
    # first chain streams right behind the DMA (subtile deps).
    w1c0 = w1_pool.tile([P, D_T, FC], bf16, tag="w1c")
    w2c0 = w2_pool.tile([P, FC_T, D], bf16, tag="w2c")
    w1cs[0], w2cs[0] = w1c0, w2c0
    MW = 2 * QW if MV1024 else QW      # GEMM1 moving width
    for di in range(D_T):
        nc.sync.dma_start(xt[:, di, ds(0, MW)], xT_r[:, di, ds(0, MW)])
    if not MV1024:
        nc.sync.dma_start(xt[:, :, ds(QW, QW)], xT_r[:, :, ds(QW, QW)])
    nc.scalar.dma_start(b1t[:], b1t_d)
    nc.scalar.dma_start(w1c0[:, :, ds(0, P)], w1_r[:, :, ds(0, P)])
    nc.scalar.dma_start(b2b[:], b2[None, :].to_broadcast((P, D)))
    nc.scalar.dma_start(w1c0[:, :, ds(P, FC - P)], w1_r[:, :, ds(P, FC - P)])
    nc.scalar.dma_start(xt[:, :, ds(2 * QW, QW)], xT_r[:, :, ds(2 * QW, QW)])
    nc.scalar.dma_start(xt[:, :, ds(3 * QW, QW)], xT_r[:, :, ds(3 * QW, QW)])
    nc.scalar.dma_start(w2c0[:], w2_r[:, ds(0, FC_T), :])

    act_fn = AFT.Tanh if ACT_FN == "tanh" else AFT.Gelu_apprx_tanh
    MW2 = 2 * QW if MV1024 else QW     # GEMM2 moving width

    for k in range(N_FC):
        # prefetch next chunk's weights early in this chunk's compute
        if k + 1 < N_FC:
            load_chunk(k + 1, nc.sync)
        w1c, w2c = w1cs[k], w2cs[k]
        w1cs[k] = w2cs[k] = None

        # ---- GEMM1: ht[f, c] = gelu(sum_d W1[d, f]^T Xt[d, c] + b1[f]) ----
        ht = ht_pool.tile([P, FC_T, C], bf16, tag="ht")
        for qi in range(C // MW):
            for fti in range(FC_T):
                ps = mm1_psum.tile([P, MW], f32, tag="mm1")
                for di in range(D_T):
                    nc.tensor.matmul(
                        ps[:],
                        lhsT=w1c[:, di, ds(fti * P, P)],
                        rhs=xt[:, di, ds(qi * MW, MW)],
                        start=(di == 0),
                        stop=(di == D_T - 1),
                    )
                ft_g = k * FC_T + fti
                nc.scalar.activation(
                    ht[:, fti, ds(qi * MW, MW)],
                    ps[:],
                    act_fn,
                    bias=b1t[:, ft_g : ft_g + 1],
                    scale=1.0,
                )

        # ---- GEMM2: Yacc[c, d] += sum_f ht[f, c]^T W2[f, d] ----
        for ci in range(C_B):
            for dci in range(D // MW2):
                ps = mm2_psum.tile([P, MW2], f32, tag="mm2")
                for fti in range(FC_T):
                    nc.tensor.matmul(
                        ps[:],
                        lhsT=ht[:, fti, ds(ci * P, P)],
                        rhs=w2c[:, fti, ds(dci * MW2, MW2)],
                        start=(fti == 0),
                        stop=(fti == FC_T - 1),
                    )
                ya = yacc[:, ci, ds(dci * MW2, MW2)]
                if k == 0:
                    nc.vector.tensor_add(
                        out=ya, in0=ps[:], in1=b2b[:, ds(dci * MW2, MW2)]
                    )
                else:
                    nc.vector.tensor_add(out=ya, in0=ya, in1=ps[:])
            if k == N_FC - 1:
                # row complete: writeback from the ACT queue (idle during
                # GEMM2 phases)
                nc.scalar.dma_start(y[ds(ci * P, P), :], yacc[:, ci, :])


_NC_CACHE = None


def build_bass():
    global _NC_CACHE
    if _NC_CACHE is not None:
        return _NC_CACHE
    nc = bacc.Bacc("TRN2", target_bir_lowering=False, debug=False)
    f32 = mybir.dt.float32
    bf16 = mybir.dt.bfloat16
    xT = nc.dram_tensor("xT", [D, C], bf16, kind="ExternalInput").ap()
    w1 = nc.dram_tensor("w1", [D, F], bf16, kind="ExternalInput").ap()
    b1t = nc.dram_tensor("b1t", [P, F // P], f32, kind="ExternalInput").ap()
    w2 = nc.dram_tensor("w2", [F, D], bf16, kind="ExternalInput").ap()
    b2 = nc.dram_tensor("b2", [D], f32, kind="ExternalInput").ap()
    y = nc.dram_tensor("y", [C, D], f32, kind="ExternalOutput").ap()
    with tile.TileContext(nc) as tc:
        with ExitStack() as ctx:
            _emit(ctx, tc, xT, w1, b1t, w2, b2, y)
    nc.compile()
    _NC_CACHE = nc
    return nc


def _in_maps(inputs, w1, b1, w2, b2):
    bf = ml_dtypes.bfloat16
    maps = []
    for e in range(E):
        xs = inputs[e * C : (e + 1) * C]
        maps.append(
            {
                "xT": np.ascontiguousarray(xs.T).astype(bf),
                "w1": w1[e].astype(bf),
                "b1t": np.ascontiguousarray(
                    b1[e].reshape(F // P, P).T.astype(np.float32)
                ),
                "w2": w2[e].astype(bf),
                "b2": np.ascontiguousarray(b2[e], dtype=np.float32),
            }
        )
    return maps


def kernel_run(inputs, w1, b1, w2, b2, trace=False, **trace_kwargs):
    """Run on 8 NeuronCores; returns (full_output [T, D], BassKernelResults)."""
    inputs = np.asarray(inputs, dtype=np.float32)
    w1 = np.asarray(w1, dtype=np.float32)
    b1 = np.asarray(b1, dtype=np.float32)
    w2 = np.asarray(w2, dtype=np.float32)
    b2 = np.asarray(b2, dtype=np.float32)
    nc = build_bass()
    res = run_bass_kernel_spmd(
        nc,
        _in_maps(inputs, w1, b1, w2, b2),
        core_ids=list(range(E)),
        trace=trace,
        **trace_kwargs,
    )
    out = np.concatenate([res.results[e]["y"] for e in range(E)], axis=0)
    return out, res


def kernel(inputs, w1, b1, w2, b2):
    out, _ = kernel_run(inputs, w1, b1, w2, b2, trace=False)
    return out


# revision 16
# speedup vs baseline: 1.1403x; 1.0164x over previous
"""Trainium2 Bass kernel for nn_LocalExperts (MoE expert-parallel FFN), v2.

Reference computation (per full input):
    x  [T=16384, D=1024] -> reshape [E=8, C=2048, D]
    h  = gelu(x @ w1[e] + b1[e])     w1 [E, D, F=4096]
    y  = h @ w2[e] + b2[e]           w2 [E, F, D]
    out[T, D]

Sharding: expert parallelism across 8 NeuronCores. Expert e's tokens are
exactly rows [e*C:(e+1)*C] of the input, so core e gets that token slice
plus w1[e], b1[e], w2[e], b2[e]. No collectives; outputs are concatenated
on the host.

v2 design (vs v1's two token passes of f32r):
  - Host prep: x arrives pre-transposed [D, C] and cast to bf16; w1/w2
    cast to bf16; b1 pre-packed [128, F/128]. No PE transposes, no
    identity, and weight DMA halves (bf16). b1/b2/PSUM/y stay f32.
  - Single token pass, F chunked in 8 x 512: weights are DMA'd exactly
    once (20 MB total reads vs v1's 76 MB).
  - bf16 matmuls get the compiler's fast-weight-load: LDWEIGHTS hides
    under the 512-row stream, unlike f32r's exposed ~187 ns loads.
  - Per chunk: GEMM1 (q-outer so chunk 0 can start as soon as the first
    quarter of Xt lands, ~4 us in), then GEMM2 starting on token blocks
    whose ht slices drained long ago -> the PE never waits on ACT.
  - Yacc [C, D] f32 accumulates GEMM2 partials on DVE; last chunk's
    drain triggers the row writeback on the ACT queue.
"""

import os
from contextlib import ExitStack

import ml_dtypes
import numpy as np

import concourse.bass as bass
import concourse.tile as tile
from concourse import bacc
from concourse import mybir
from concourse.bass import ds, ts
from concourse.bass_utils import run_bass_kernel_spmd

AFT = mybir.ActivationFunctionType

E = 8
D = 1024
F = 4096
T = 16384
C = T // E          # tokens per core
P = 128

FC = 512            # F chunk per iteration
N_FC = F // FC      # 8 chunks
FC_T = FC // P      # 4 f-tiles per chunk
D_T = D // P        # 8 d-tiles
C_B = C // P        # 16 token blocks
NQ = 4              # token quarters for GEMM1 moving dim
QW = C // NQ        # 512 tokens per quarter
N_WARM = int(os.environ.get("KERNEL_WARM", "5"))  # HAM clock warm-up matmuls
# moving free dim 1024 (2 PSUM banks/matmul, halves instruction count);
# bf16 moving operand max is 128x1024 per the TRN2 ISA table
MV1024 = os.environ.get("KERNEL_MV1024", "0") == "1"

# test-only: CoreSim lacks Gelu; "tanh" swaps the activation for sim gating
ACT_FN = os.environ.get("KERNEL_ACT", "gelu")


def _emit(ctx: ExitStack, tc: tile.TileContext, xT, w1, b1t_d, w2, b2, y):
    nc = tc.nc
    f32 = mybir.dt.float32
    bf16 = mybir.dt.bfloat16

    consts = ctx.enter_context(tc.tile_pool(name="consts", bufs=1))
    xt_pool = ctx.enter_context(tc.tile_pool(name="xt", bufs=1))
    yacc_pool = ctx.enter_context(tc.tile_pool(name="yacc", bufs=1))
    w1_pool = ctx.enter_context(tc.tile_pool(name="w1c", bufs=2))
    w2_pool = ctx.enter_context(tc.tile_pool(name="w2c", bufs=3))
    ht_pool = ctx.enter_context(tc.tile_pool(name="ht", bufs=3))
    mm1_psum = ctx.enter_context(tc.tile_pool(name="mm1", bufs=2, space="PSUM"))
    mm2_psum = ctx.enter_context(
        tc.tile_pool(name="mm2", bufs=(2 if MV1024 else 4), space="PSUM")
    )

    # Warm the PE HAM clock (cold 1.2GHz -> 2.4GHz needs ~3.4us of activity)
    # while the first Xt slices + w1 chunk DMA in; the first real chain is
    # DMA-paced, so it continues the busy stream and finishes the ramp.
    dummy = consts.tile([P, QW], bf16)
    nc.gpsimd.memset(dummy[:], 0.0)
    for _ in range(N_WARM):
        warm_ps = mm1_psum.tile([P, QW], f32, tag="mm1", name="warm_ps")
        nc.tensor.matmul(warm_ps[:], lhsT=dummy[:, :P], rhs=dummy[:],
                         start=True, stop=True)

    # ---- staged inputs ----
    xT_r = xT.rearrange("(do p) c -> p do c", p=P)    # [128, 8, 2048]
    w1_r = w1.rearrange("(do p) f -> p do f", p=P)    # [128, 8, 4096]
    w2_r = w2.rearrange("(fo p) d -> p fo d", p=P)    # [128, 32, 1024]

    xt = xt_pool.tile([P, D_T, C], bf16, tag="xt")
    yacc = yacc_pool.tile([P, C_B, D], f32, tag="yacc")
    b1t = consts.tile([P, F // P], f32)
    b2b = consts.tile([P, D], f32)

    w1cs = [None] * N_FC
    w2cs = [None] * N_FC

    def load_chunk(k, q):
        w1cs[k] = w1_pool.tile([P, D_T, FC], bf16, tag="w1c", name=f"w1c{k}")
        w2cs[k] = w2_pool.tile([P, FC_T, D], bf16, tag="w2c", name=f"w2c{k}")
        q.dma_start(w1cs[k][:], w1_r[:, :, ds(k * FC, FC)])
        q.dma_start(w2cs[k][:], w2_r[:, ds(k * FC_T, FC_T), :])

    # Startup DMAs. The scalar (ACT) queue gets ONLY the two tiny loads
    # the first GEMM1 drain depends on (b1t bias + w1 chunk 0's first
    # f-tile) so the first ACTIVATE isn't stuck behind bulk DMA issues;
    # everything else streams on the sync queue. Xt quarter 0 lands in
    # di-pair slices so the first chain can stream behind the DMA.
    w1c0 = w1_pool.tile([P, D_T, FC], bf16, tag="w1c")
    w2c0 = w2_pool.tile([P, FC_T, D], bf16, tag="w2c")
    w1cs[0], w2cs[0] = w1c0, w2c0
    nc.scalar.dma_start(b1t[:], b1t_d)
    nc.scalar.dma_start(w1c0[:, :, ds(0, P)], w1_r[:, :, ds(0, P)])
    for di2 in range(D_T // 2):
        nc.sync.dma_start(
            xt[:, ds(2 * di2, 2), ds(0, QW)], xT_r[:, ds(2 * di2, 2), ds(0, QW)]
        )
    nc.sync.dma_start(w1c0[:, :, ds(P, FC - P)], w1_r[:, :, ds(P, FC - P)])
    nc.sync.dma_start(xt[:, :, ds(QW, QW)], xT_r[:, :, ds(QW, QW)])
    nc.sync.dma_start(xt[:, :, ds(2 * QW, QW)], xT_r[:, :, ds(2 * QW, QW)])
    nc.sync.dma_start(xt[:, :, ds(3 * QW, QW)], xT_r[:, :, ds(3 * QW, QW)])
    nc.sync.dma_start(b2b[:], b2[None, :].to_broadcast((P, D)))
    nc.sync.dma_start(w2c0[:], w2_r[:, ds(0, FC_T), :])

    act_fn = AFT.Tanh if ACT_FN == "tanh" else AFT.Gelu_apprx_tanh

    hts = {}
    for k in range(N_FC):
        # prefetch next chunk's weights early in this chunk's compute
        if k + 1 < N_FC:
            load_chunk(k + 1, nc.sync)
        w1c = w1cs[k]
        w1cs[k] = None

        # ---- GEMM1: ht[f, c] = gelu(sum_d W1[d, f]^T Xt[d, c] + b1[f]) ----
        ht = ht_pool.tile([P, FC_T, C], bf16, tag="ht", name=f"ht{k}")
        hts[k] = ht
        for qi in range(NQ):
            for fti in range(FC_T):
                ps = mm1_psum.tile([P, QW], f32, tag="mm1")
                for di in range(D_T):
                    nc.tensor.matmul(
                        ps[:],
                        lhsT=w1c[:, di, ds(fti * P, P)],
                        rhs=xt[:, di, ds(qi * QW, QW)],
                        start=(di == 0),
                        stop=(di == D_T - 1),
                    )
                ft_g = k * FC_T + fti
                nc.scalar.activation(
                    ht[:, fti, ds(qi * QW, QW)],
                    ps[:],
                    act_fn,
                    bias=b1t[:, ft_g : ft_g + 1],
                    scale=1.0,
                )

        # ---- GEMM2 on chunk pairs: Yacc[c, d] += sum_f ht^T W2 over the
        # pair's 8 f-tiles in one PSUM chain (halves the DVE drain count,
        # which otherwise rate-matches the PE and beats against it) ----
        if k % 2 == 0:
            continue
        pair = (k - 1, k)
        for ci in range(C_B):
            for dci in range(2):
                ps = mm2_psum.tile([P, QW], f32, tag="mm2")
                for kk in pair:
                    for fti in range(FC_T):
                        nc.tensor.matmul(
                            ps[:],
                            lhsT=hts[kk][:, fti, ds(ci * P, P)],
                            rhs=w2cs[kk][:, fti, ds(dci * QW, QW)],
                            start=(kk == pair[0] and fti == 0),
                            stop=(kk == pair[1] and fti == FC_T - 1),
                        )
                ya = yacc[:, ci, ds(dci * QW, QW)]
                if k == 1:
                    nc.vector.tensor_add(
                        out=ya, in0=ps[:], in1=b2b[:, ds(dci * QW, QW)]
                    )
                else:
                    nc.vector.tensor_add(out=ya, in0=ya, in1=ps[:])
            if k == N_FC - 1:
                # row complete: writeback from the ACT queue (idle during
                # GEMM2 phases)
                nc.scalar.dma_start(y[ds(ci * P, P), :], yacc[:, ci, :])
        hts[k - 1] = hts[k] = None
        w2cs[k - 1] = w2cs[k] = None


_NC_CACHE = None


def build_bass():
    global _NC_CACHE
    if _NC_CACHE is not None:
        return _NC_CACHE
    nc = bacc.Bacc("TRN2", target_bir_lowering=False, debug=False)
    f32 = mybir.dt.float32
    bf16 = mybir.dt.bfloat16
    xT = nc.dram_tensor("xT", [D, C], bf16, kind="ExternalInput").ap()
    w1 = nc.dram_tensor("w1", [D, F], bf16, kind="ExternalInput").ap()
    b1t = nc.dram_tensor("b1t", [P, F // P], f32, kind="ExternalInput").ap()
    w2 = nc.dram_tensor("w2", [F, D], bf16, kind="ExternalInput").ap()
    b2 = nc.dram_tensor("b2", [D], f32, kind="ExternalInput").ap()
    y = nc.dram_tensor("y", [C, D], f32, kind="ExternalOutput").ap()
    with tile.TileContext(nc) as tc:
        with ExitStack() as ctx:
            _emit(ctx, tc, xT, w1, b1t, w2, b2, y)
    nc.compile()
    _NC_CACHE = nc
    return nc


def _in_maps(inputs, w1, b1, w2, b2):
    bf = ml_dtypes.bfloat16
    maps = []
    for e in range(E):
        xs = inputs[e * C : (e + 1) * C]
        maps.append(
            {
                "xT": np.ascontiguousarray(xs.T).astype(bf),
                "w1": w1[e].astype(bf),
                "b1t": np.ascontiguousarray(
                    b1[e].reshape(F // P, P).T.astype(np.float32)
                ),
                "w2": w2[e].astype(bf),
                "b2": np.ascontiguousarray(b2[e], dtype=np.float32),
            }
        )
    return maps


def kernel_run(inputs, w1, b1, w2, b2, trace=False, **trace_kwargs):
    """Run on 8 NeuronCores; returns (full_output [T, D], BassKernelResults)."""
    inputs = np.asarray(inputs, dtype=np.float32)
    w1 = np.asarray(w1, dtype=np.float32)
    b1 = np.asarray(b1, dtype=np.float32)
    w2 = np.asarray(w2, dtype=np.float32)
    b2 = np.asarray(b2, dtype=np.float32)
    nc = build_bass()
    res = run_bass_kernel_spmd(
        nc,
        _in_maps(inputs, w1, b1, w2, b2),
        core_ids=list(range(E)),
        trace=trace,
        **trace_kwargs,
    )
    out = np.concatenate([res.results[e]["y"] for e in range(E)], axis=0)
    return out, res


def kernel(inputs, w1, b1, w2, b2):
    out, _ = kernel_run(inputs, w1, b1, w2, b2, trace=False)
    return out
